# revision 39
# baseline (speedup 1.0000x reference)
"""CBOW negative-sampling loss kernel for Trainium2 (8 NeuronCores).

Strategy: data-parallel over batch (16384 -> 8 x 2048). Each batch row
needs 31 embedding rows (10 ctx + 1 center + 20 neg) of 300 f32. The
v1 baseline gathered them with per-slot SWDGE indirect DMAs (one index
per output partition -> 496 Pool instructions/core at ~1.6 us each =
~774 us, Pool-descriptor-generation bound). This version uses the
custom `dma_gather` SWDGE instruction (InstDMAGatherAnt, mlp library,
auto-loaded by insert_library_loads), which gathers up to 1024 indices
in ONE Pool instruction. Rows are fetched in PAIRS -- the host dedups
(slotA,slotB) vocab-row pairs into sub-table entries holding both rows
back-to-back (1216 B payload, 1536 B stride), so each tile needs 16
descriptors per batch row instead of 31 and the SDMA drain (~40 MB/core
random 1216 B reads) stays below the DVE compute (~165 us), which is
the final critical path. Measured ~185 us/core (4.2x over baseline).

dma_gather facts this kernel is built around (all HW-verified):
  - int16 indices only: vocab is 100k, but each half-core (8 tiles =
    16384 pair draws) touches at most 16384 < 32768 unique pairs, so
    the host dedups pair keys per half and uploads a per-half fused
    bf16 pair sub-table plus remapped int16 codes. Slot 30 pairs with
    an all-zero sentinel row whose half is layout-only (last cn slot,
    excluded from all compute slices). Device traffic is unchanged;
    only the naming is compacted. Mid-stream negative indices are sign-extended into
    wild addresses (only a trailing -1 run is trimmed) -- never pad
    with -1.
  - the runtime SWDGE descriptor ring holds ~64 data descriptors per
    SDMA lane, so one gather is capped at 1024 indices (65 descs/lane
    incl. the sem descriptor); 1152+ hangs the decode-side await_space
    forever (the "dma_gather faults on HW" of the v1 notes). Each tile
    therefore uses 3 gathers (5 ctx pairs / 8 + 3 cn pairs) rotating
    the 4 SWDGE queues (num_swdge_queues=4) so one queue's descriptor
    generation overlaps other queues' drains, and so the ctx-sum tree
    only waits on its own small gather.
  - row STRIDE must be a multiple of 256 B, but the per-index payload
    need not be: pair entries are bf16 at 1536 B pitch (768 cols) and
    each descriptor moves only the 1200 B of real data (dma_gather_raw
    skips bass's elem%256 assert, which only the transpose path needs).
  - index layout: index i of a gather lives at wrapped[i%16, i//16] in
    SBUF, read from partitions 32q..32q+31 for queue q -> the host
    replicates the [16, w] block across all 128 partitions.
  - destination: gathered row i -> out[i%128, i//128, :], so ordering
    i = slot_local*128 + p puts (batch row p, slot s) at partition p,
    free slot s -- exactly the layout the per-partition compute needs.

Compute per tile, all dense step-1 bf16 so DVE packs 2 elem/cycle
(tensor_reduce has only a 1x uop; tensor_scalar/copy would auto-select
the 2-port 4x mode and stall multi-us against the SWDGE descriptor
rings' SBUF traffic, and tensor_tensor_reduce faults on HW, so the
kernel sticks to tensor_tensor + small 1x reduces):
  - ctx_sum: pairwise tensor_tensor add tree over the 10 ctx slots.
  - prod = cn_embs * ctx_sum (in1 broadcast over the 21 real cn
    slots; the zero pair half is never touched).
  - compute runs at width DC=304 (4 zero pad cols): 304 = 16*19, so
    four more pairwise adds halve d 304->152->76->38->19 before the
    1x-only reduce, which then reads only 399 elems per tile for the
    raw scores [128, 21] f32.
Tail (once per core): scores *= [-0.1, +0.1*20] (folds the score scale
and the log-sigmoid sign), one Exp over [128, 336], one Ln(1+x), one
DVE reduce over the 21 slots -> per-row losses [128, 16]; the host
sums them. Exp/Ln run back-to-back so ACT loads each function table
exactly once.
"""

import sys

for _p in ("/opt/trn_rl_repo", "/root/.axon_site/_ro/trn_rl_repo"):
    if _p not in sys.path:
        sys.path.append(_p)

import numpy as np

VOCAB = 100000
D = 300
DC = 304  # compute width: 4 zero pad cols make d 16*19, so the pairwise
          # add tree halves 4 times (304->152->76->38->19) before the
          # 1x-only tensor_reduce, which then reads 418 elems not 1650
DPAD = 384  # pair sub-table pitch is 2*DPAD bf16 cols = 1536 B
N_CTX = 10
N_NEG = 20
N_CN = 1 + N_NEG  # 21
N_SLOTS = N_CTX + N_CN  # 31
N_CORES = 8
BATCH = 16384
P = 128
B_CORE = BATCH // N_CORES  # 2048
N_TILES = B_CORE // P  # 16
N_PS = 16  # pair-slots per batch row: (0,1)..(28,29),(30,zero)
N_SC = N_CN + 1  # 22 score cols per tile (last is the zero pair half)
SENT = 2 * VOCAB  # sentinel key = the all-zero row of the fused wall
IDX_PER_TILE = N_PS * P  # 2048 pair-gathers per tile
IDX_W = IDX_PER_TILE // 16  # 128 int16 per partition per tile
U_MAX = (N_TILES // 2) * P * N_PS  # 16384 pair entries per half
# pair-slot groups per gather instruction (<=1024 idxs = ring capacity).
# Group 0 (the 5 ctx pairs) fills its own tile so the ctx-sum tree can
# start after one small gather; groups 1-2 fill the cn tile.
PAIR_GROUPS = ((0, 5), (5, 13), (13, N_PS))


def dma_gather_raw(
    nc, out_ap, in_ap, idxs_ap, num_idxs, elem_size, queue_num=0
):
    """bass.dma_gather minus the elem_size_bytes%256 assert: that check is a
    transpose-mode restriction misapplied to the non-transpose path (the
    firmware builds one descriptor of elem_size bytes per index; only the
    row STRIDE must be a multiple of 256B). Lets us gather 600B payloads
    from 768B-strided rows."""
    from concourse import mybir
    from concourse.bass import ap_utils

    g = nc.gpsimd
    g._assert_queue_num(queue_num)
    assert idxs_ap.dtype == mybir.dt.int16
    assert in_ap.dtype == out_ap.dtype
    assert ap_utils.ap_is_contiguous(in_ap.ap[1:])
    assert ap_utils.ap_is_contiguous(out_ap.ap[1:])
    assert ap_utils.ap_is_contiguous(idxs_ap.ap[1:])
    assert in_ap.ap[-1][1] == out_ap.ap[-1][1] == elem_size
    assert out_ap.ap[0][1] * out_ap.ap[1][1] == ((num_idxs + 127) // 128) * 128
    elem_step = in_ap.ap[0][0]
    stride_bytes = elem_step * mybir.dt.size(in_ap.dtype)
    stride_bytes_256 = stride_bytes // 256
    assert stride_bytes % 256 == 0 and 0 < stride_bytes_256 < 256
    _in_ap = g.lower_ap_dma(in_ap, for_custom_bir_dma=True)
    _idxs_ap = g.lower_ap(idxs_ap)
    _out_ap = g.lower_ap(out_ap)
    return g.add_instruction(
        mybir.InstDMAGatherAnt(
            name=nc.get_next_instruction_name(),
            ins=[*_in_ap, _idxs_ap, g.lower_val_access(g.to_reg(num_idxs))],
            outs=[_out_ap],
            transpose=False,
            num_idxs=num_idxs,
            elem_size=elem_size,
            stride_bytes_256=stride_bytes_256,
            gen_mode=0,
            single_packet=True,
            queue_num=queue_num,
            sbuf_tokens_per_rank=0,
            sbuf_free_dim_per_rank=0,
            sbuf_free_dim_pad_per_rank=0,
            sbuf_byte_offset=0,
        )
    )


def emit_cbow_body(nc, tc, idx16, sub0, sub1, signs, out, n_tiles):
    """Emit the per-core program body into an open TileContext.

    idx16: [P, n_tiles*IDX_W] int16 DRAM (remapped, wrapped, replicated)
    sub0:  [u_max, 768] bf16 DRAM -- fused PAIR sub-table, tiles 0..h-1
           (cols 0:300 = first row, 300:600 = second row, 600:768 pad)
    sub1:  [u_max, 768] bf16 DRAM -- same for tiles h..n-1
    signs: [P, N_CN] f32 DRAM -- [-0.1, +0.1 x20] replicated rows
    out:   [P, n_tiles] f32 DRAM -- out[p, t] = sum_i ln(1+exp(-x_i))
    """
    from concourse import mybir

    f32 = mybir.dt.float32
    bf16 = mybir.dt.bfloat16
    i16 = mybir.dt.int16
    add = mybir.AluOpType.add
    mult = mybir.AluOpType.mult
    half = n_tiles // 2
    with (
        tc.tile_pool(name="gctx", bufs=4) as gcpool,
        tc.tile_pool(name="gcn", bufs=4) as gnpool,
        tc.tile_pool(name="small", bufs=3) as spool,
        tc.tile_pool(name="accp", bufs=1) as apool,
    ):
        acc = apool.tile([P, n_tiles], f32)
        # raw scores for every tile: col t*N_CN+0 = pos_score, cols
        # 1..20 = neg_scores (the zero pair half never enters scores).
        sc_all = apool.tile([P, n_tiles * N_CN], f32)
        idx_sb = apool.tile([P, n_tiles * IDX_W], i16)
        # tile 0's columns first so its gathers can issue immediately
        nc.sync.dma_start(out=idx_sb[:, :IDX_W], in_=idx16[:, :IDX_W])
        if n_tiles > 1:
            nc.sync.dma_start(out=idx_sb[:, IDX_W:], in_=idx16[:, IDX_W:])
        signs_sb = apool.tile([P, N_CN], f32)
        nc.sync.dma_start(out=signs_sb[:], in_=signs[:])
        n_gather = 0
        for t in range(n_tiles):
            sub = sub0 if t < half else sub1
            # pair-slot k holds (slot 2k, slot 2k+1): gc = slots 0..9,
            # gn = slots 10..30 + a zero column.
            gc = gcpool.tile([P, N_CTX * DC], bf16, tag="gc")
            gn = gnpool.tile([P, N_SC * DC], bf16, tag="gn")
            # The runtime SWDGE descriptor ring holds only ~64 data
            # descriptors per SDMA lane, so one gather is capped at 1024
            # indices (65 descs/lane incl. the sem). Rotate the 4 SWDGE
            # queues so one queue's descriptor generation overlaps
            # another queue's DMA drain.
            col = t * IDX_W
            for s0, s1 in PAIR_GROUPS:
                w = (s1 - s0) * P // 16
                if s1 <= 5:
                    out_ap = gc[:, s0 * 2 * DC : s1 * 2 * DC]
                else:
                    out_ap = gn[:, (s0 - 5) * 2 * DC : (s1 - 5) * 2 * DC]
                dma_gather_raw(
                    nc,
                    out_ap=out_ap.rearrange("p (s d) -> p s d", s=s1 - s0),
                    in_ap=sub[:, : 2 * DC],
                    idxs_ap=idx_sb[:, col : col + w],
                    num_idxs=(s1 - s0) * P,
                    elem_size=2 * DC,
                    queue_num=n_gather % 4,
                )
                col += w
                n_gather += 1

            # ctx_sum[p, :] = sum of slots 0..9 (pairwise tree, dense bf16)
            c1 = spool.tile([P, 5 * DC], bf16, tag="c1")
            nc.vector.tensor_tensor(
                out=c1[:], in0=gc[:, : 5 * DC], in1=gc[:, 5 * DC :], op=add
            )
            c2 = spool.tile([P, 2 * DC], bf16, tag="c2")
            nc.vector.tensor_tensor(
                out=c2[:], in0=c1[:, : 2 * DC], in1=c1[:, 2 * DC : 4 * DC],
                op=add,
            )
            c3 = spool.tile([P, DC], bf16, tag="c3")
            nc.vector.tensor_tensor(
                out=c3[:], in0=c2[:, :DC], in1=c2[:, DC : 2 * DC], op=add
            )
            ctx = spool.tile([P, DC], bf16, tag="ctx")
            nc.vector.tensor_tensor(
                out=ctx[:], in0=c3[:], in1=c1[:, 4 * DC : 5 * DC], op=add
            )

            # prod[p, n, d] = cn[p, n, d] * ctx[p, d]; then a pairwise-add
            # tree halves d before the (1x-only) reduce: tensor_reduce has
            # no packed uop, so shrink its input with 2x tensor_tensor
            # adds first. (The +-0.1 sign/scale is applied to the [P, 336]
            # scores at the end -- a tensor_scalar here would auto-select
            # the 2-port 4x mode and stall multi-us against the SWDGE
            # descriptor-ring SBUF traffic of the concurrent gathers.)
            prod = spool.tile([P, N_CN * DC], bf16, tag="prod")
            nc.vector.tensor_tensor(
                out=prod.rearrange("p (n d) -> p n d", n=N_CN),
                in0=gn[:, : N_CN * DC].rearrange(
                    "p (n d) -> p n d", n=N_CN
                ),
                in1=ctx.unsqueeze(1).broadcast_to([P, N_CN, DC]),
                op=mult,
            )
            hw_ = DC
            hin = prod
            for lvl in range(4):
                hw_ //= 2
                hout = spool.tile([P, N_CN * hw_], bf16, tag=f"h{lvl}")
                hv = hin.rearrange("p (n d) -> p n d", n=N_CN)
                nc.vector.tensor_tensor(
                    out=hout.rearrange("p (n d) -> p n d", n=N_CN),
                    in0=hv[:, :, 0:hw_],
                    in1=hv[:, :, hw_ : 2 * hw_],
                    op=add,
                )
                hin = hout
            nc.vector.tensor_reduce(
                out=sc_all[:, t * N_CN : (t + 1) * N_CN],
                in_=hin.rearrange("p (n d) -> p n d", n=N_CN),
                axis=mybir.AxisListType.X,
                op=add,
            )
        # Apply the +-0.1 sign/scale to all raw scores at once (the accum
        # target is -x_n: center gets -0.1, negatives +0.1), then the
        # softplus tail, batched so ACT loads each function table once:
        # acc[:, t] = sum_n ln(1 + exp(sc_all[:, t, n])).
        nc.vector.tensor_tensor(
            out=sc_all.rearrange("p (t n) -> p t n", t=n_tiles),
            in0=sc_all.rearrange("p (t n) -> p t n", t=n_tiles),
            in1=signs_sb.unsqueeze(1).broadcast_to([P, n_tiles, N_CN]),
            op=mult,
        )
        ex_all = apool.tile([P, n_tiles * N_CN], f32)
        nc.scalar.activation(
            out=ex_all[:],
            in_=sc_all[:],
            func=mybir.ActivationFunctionType.Exp,
        )
        ln_all = apool.tile([P, n_tiles * N_CN], f32)
        nc.scalar.activation(
            out=ln_all[:],
            in_=ex_all[:],
            func=mybir.ActivationFunctionType.Ln,
            bias=1.0,
        )
        nc.vector.tensor_reduce(
            out=acc[:],
            in_=ln_all.rearrange("p (t n) -> p t n", t=n_tiles),
            axis=mybir.AxisListType.X,
            op=add,
        )
        nc.sync.dma_start(out=out[:], in_=acc[:])


def build_program(n_tiles=N_TILES, u_max=U_MAX, n_cores=N_CORES):
    from concourse import mybir
    import concourse.bacc as bacc
    import concourse.tile as tile

    nc = bacc.Bacc(
        "TRN2",
        target_bir_lowering=False,
        debug=False,
        enable_asserts=False,
        num_devices=n_cores,
        num_swdge_queues=4,
    )
    idx16 = nc.dram_tensor(
        "idx16", [P, n_tiles * IDX_W], mybir.dt.int16, kind="ExternalInput"
    ).ap()
    sub0 = nc.dram_tensor(
        "sub0", [u_max, 2 * DPAD], mybir.dt.bfloat16, kind="ExternalInput"
    ).ap()
    sub1 = nc.dram_tensor(
        "sub1", [u_max, 2 * DPAD], mybir.dt.bfloat16, kind="ExternalInput"
    ).ap()
    signs = nc.dram_tensor(
        "signs", [P, N_CN], mybir.dt.float32, kind="ExternalInput"
    ).ap()
    out = nc.dram_tensor(
        "out", [P, n_tiles], mybir.dt.float32, kind="ExternalOutput"
    ).ap()
    with tile.TileContext(nc) as tc:
        emit_cbow_body(nc, tc, idx16, sub0, sub1, signs, out, n_tiles)
    nc.compile()
    return nc


_NC_CACHE = {}


def _get_program():
    if "nc" not in _NC_CACHE:
        _NC_CACHE["nc"] = build_program()
    return _NC_CACHE["nc"]


def pack_keys(context, center, negatives):
    """[BATCH, N_SLOTS] int32 fused keys: ctx rows (cols 0..9) keep their
    vocab id; center/neg rows (cols 10..30) get +VOCAB (cen_w table)."""
    ctx = np.asarray(context, dtype=np.int32).reshape(BATCH, N_CTX)
    cen = np.asarray(center, dtype=np.int32).reshape(BATCH, 1) + VOCAB
    neg = np.asarray(negatives, dtype=np.int32).reshape(BATCH, N_NEG) + VOCAB
    return np.ascontiguousarray(np.concatenate([ctx, cen, neg], axis=1))


def build_wall(context_weight, center_weight):
    """[2*VOCAB+1, D] bf16 fused weight table; last row = zero sentinel."""
    import ml_dtypes

    wall = np.zeros((2 * VOCAB + 1, DC), dtype=ml_dtypes.bfloat16)
    wall[:VOCAB, :D] = np.asarray(context_weight, dtype=np.float32)
    wall[VOCAB : 2 * VOCAB, :D] = np.asarray(center_weight, dtype=np.float32)
    return wall


def prepare_core_inputs(keys_core, wall, n_tiles=N_TILES, u_max=U_MAX):
    """Build one core's device inputs.

    keys_core: [n_tiles*P, N_SLOTS] int32 fused keys (batch-tile order:
        row t*P+p -> tile t, partition p).
    wall: [2*VOCAB, DPAD] bf16 padded fused table.
    Returns dict(idx16=[P, n_tiles*IDX_W] i16, sub0, sub1=[u_max, DPAD] bf16).
    """
    import ml_dtypes

    half = n_tiles // 2
    subs = []
    wrapped_cols = []
    # pair slots (0,1),(2,3)...(28,29),(30,SENT): one gather descriptor
    # fetches both rows of a pair (1200 B) -- half the descriptors of a
    # per-row gather for the same bytes.
    a_cols = list(range(0, N_SLOTS - 1, 2)) + [N_SLOTS - 1]
    b_cols = list(range(1, N_SLOTS, 2))
    for h in (0, 1):
        blk = keys_core[h * half * P : (h + 1) * half * P]  # [half*P, N_SLOTS]
        ka = blk[:, a_cols].astype(np.int64)  # [half*P, N_PS]
        kb = np.concatenate(
            [
                blk[:, b_cols].astype(np.int64),
                np.full((blk.shape[0], 1), SENT, dtype=np.int64),
            ],
            axis=1,
        )
        pk = ka * (SENT + 1) + kb
        uniq, inv = np.unique(pk, return_inverse=True)
        assert len(uniq) <= u_max
        pa = (uniq // (SENT + 1)).astype(np.int64)
        pb = (uniq % (SENT + 1)).astype(np.int64)
        sub = np.zeros((u_max, 2 * DPAD), dtype=ml_dtypes.bfloat16)
        sub[: len(uniq), 0:DC] = wall[pa]
        sub[: len(uniq), DC : 2 * DC] = wall[pb]
        subs.append(sub)
        inv16 = inv.astype(np.int16).reshape(half, P, N_PS)
        for tt in range(half):
            for s0, s1 in PAIR_GROUPS:
                flat = inv16[tt].T[s0:s1].ravel()  # i = (s-s0)*P + p
                wrapped_cols.append(flat.reshape(len(flat) // 16, 16).T)
    idx16 = np.tile(np.concatenate(wrapped_cols, axis=1), (P // 16, 1))
    return {
        "idx16": np.ascontiguousarray(idx16),
        "sub0": subs[0],
        "sub1": subs[1],
        "signs": np.tile(
            np.array([[-0.1] + [0.1] * N_NEG], dtype=np.float32), (P, 1)
        ),
    }


def make_in_maps(context, center, negatives, context_weight, center_weight):
    keys = pack_keys(context, center, negatives).reshape(
        N_CORES, B_CORE, N_SLOTS
    )
    wall = build_wall(context_weight, center_weight)
    return [prepare_core_inputs(keys[c], wall) for c in range(N_CORES)]


def kernel(context, center, negatives, context_weight, center_weight):
    from concourse import bass_utils

    nc = _get_program()
    in_maps = make_in_maps(
        context, center, negatives, context_weight, center_weight
    )
    res = bass_utils.run_bass_kernel_spmd(nc, in_maps, core_ids=list(range(N_CORES)))
    acc = np.stack([r["out"] for r in res.results])  # [N_CORES, P, N_TILES]
    return np.array(acc.sum(dtype=np.float64) / BATCH, dtype=np.float32)
